# revision 21
# baseline (speedup 1.0000x reference)
"""BertSelfAttention (B=4, S=2048, H=1024, NH=16, HD=64) on 8 Trainium2 NeuronCores.

Sharding: batch (4) x head-group (2) -> 8 cores. Core c handles batch b=c//2 and
heads [g*8, g*8+8) with g=c%2 (output channels [g*512, (g+1)*512)).

Per-core math (all on device):
  QT[ch, s] = (wq_c @ x_b^T + bq_c),  KT likewise       (channels on partitions)
  V[s, ch]  = (x_b @ wv_c^T + bv_c)                     (tokens on partitions)
  per (head h, query half ih), per key tile st (128 keys j):
      scoresT[j, i] -> [128, 1024] PSUM (2 matmuls), ping-pong buffered
      expT = exp(scoresT/8 + mask_j)   (one ACT op; mask is per-partition bias)
      ctxT[d, i] += [v_h | 1]^T-weighted expT           (fused denominator row)
  Device emits unnormalized ctxT + denom rows [8*65, 2048]; the host divides and
  transposes into [B, S, H].

Two Trainium2-specific tricks matter here:
  * Changing the matmul contraction size (K) between back-to-back matmuls costs
    ~1.6us in PE reconfiguration, so every matmul keeps K=128: Q is stored
    per-head zero-padded to 128 partitions (the other head's K rows hit zeros),
    while KT stays packed two heads per tile.
  * All PSUM lives in one pool of 4 [128, 1024] tags: QKV passes use tile
    halves as 8 accumulators, attention ping-pongs scores on tags 0/1 and ctx
    on tags 2/3 -- no pool-transition barrier or head-boundary PE stalls.

Matmuls run as float32r (full-rate fp32 with hardware rounding, ~2e-4 rel err).
"""

import os
import sys

if "/opt/trn_rl_repo" not in sys.path:
    sys.path.insert(0, "/opt/trn_rl_repo")

import numpy as np

_KERNEL_DIR = os.path.dirname(os.path.abspath(__file__))

B, S, H = 4, 2048, 1024
NH, HD = 16, 64
HPC = 8          # heads per core
CH = HPC * HD    # 512 output channels per core
CT = H // 128    # 8 contraction tiles
JT = CH // 128   # 4 channel tiles per core
ST = S // 128    # 16 token tiles
VW = HD + 1      # 65: v columns + fused ones column

_CACHE = {}


def _build():
    import concourse.bass as bass  # noqa: F401  (registers engine methods)
    import concourse.mybir as mybir
    import concourse.tile as tile
    from concourse import bacc

    F32 = mybir.dt.float32
    F32R = mybir.dt.float32r

    nc = bacc.Bacc("TRN2", target_bir_lowering=False, debug=True)

    xt = nc.dram_tensor("xt", [H, S], F32, kind="ExternalInput")        # x_b^T
    wq_t = nc.dram_tensor("wq_t", [H, CH], F32, kind="ExternalInput")   # wq_c^T
    wk_t = nc.dram_tensor("wk_t", [H, CH], F32, kind="ExternalInput")
    wv_t = nc.dram_tensor("wv_t", [H, CH], F32, kind="ExternalInput")
    bq = nc.dram_tensor("bq", [CH], F32, kind="ExternalInput")
    bk = nc.dram_tensor("bk", [CH], F32, kind="ExternalInput")
    bv = nc.dram_tensor("bv", [CH], F32, kind="ExternalInput")
    mask = nc.dram_tensor("mask", [S], F32, kind="ExternalInput")
    ones = nc.dram_tensor("ones", [512], F32, kind="ExternalInput")
    # unnormalized ctxT + denominator rows, 65 rows per head
    out = nc.dram_tensor("out", [VW * HPC, S], F32, kind="ExternalOutput")

    with tile.TileContext(nc) as tc, nc.allow_low_precision(reason="fp32r attention"):
        from contextlib import ExitStack

        with ExitStack() as outer:
            persist = outer.enter_context(tc.tile_pool(name="persist", bufs=1))
            ppool = outer.enter_context(tc.tile_pool(name="pp", bufs=1, space="PSUM"))

            # Persistent SBUF tensors
            # Q per head, zero-padded to 128 partitions (head h lives in its own
            # partition range po:po+64; the other 64 rows are zeros).
            qp_sb = [persist.tile([128, S], F32R, tag=f"qp{h}", name=f"qp{h}")
                     for h in range(HPC)]
            kt_sb = [persist.tile([128, S], F32R, tag=f"kt{j}", name=f"kt{j}")
                     for j in range(JT)]
            v_sb = persist.tile([128, ST, VW * HPC], F32R, tag="v")
            mask_sb = persist.tile([128, ST], F32, tag="mask")
            bqp = persist.tile([128, JT], F32, tag="bqp")
            bkp = persist.tile([128, JT], F32, tag="bkp")
            bv_bc = persist.tile([128, CH], F32, tag="bv_bc")
            ones8 = persist.tile([128, HPC], F32R, tag="ones8")
            zcol = persist.tile([128, 1], F32, tag="zcol")

            nc.sync.dma_start(out=mask_sb, in_=mask.rearrange("(t p) -> p t", p=128))
            nc.sync.dma_start(out=bqp, in_=bq.rearrange("(j p) -> p j", p=128))
            nc.sync.dma_start(out=bkp, in_=bk.rearrange("(j p) -> p j", p=128))
            nc.sync.dma_start(
                out=bv_bc,
                in_=bass.AP(tensor=bv, offset=0, ap=[[0, 128], [1, CH]]))
            nc.sync.dma_start(
                out=ones8,
                in_=bass.AP(tensor=ones.bitcast(F32R), offset=0,
                            ap=[[0, 128], [1, HPC]]))
            # ones columns of v (position 64 of each head block, every token tile)
            v4 = v_sb.rearrange("p t (h e) -> p t h e", e=VW)
            for t in range(ST):
                nc.vector.tensor_copy(v4[:, t, :, HD], ones8)
            # zero the unused partition half of each padded-Q tile
            nc.vector.memset(zcol, 0.0)
            for h in range(HPC):
                zo = 64 if h % 2 == 0 else 0      # rows NOT owned by head h
                zsrc = zcol[zo:zo + 64, 0:1]
                zbcast = bass.AP(tensor=zsrc.tensor, offset=zsrc.offset,
                                 ap=[zsrc.ap[0], [0, S]])
                nc.vector.tensor_copy(qp_sb[h][zo:zo + 64, :], zbcast)

            # ---------------- Phase 1: QKV projections ----------------
            with ExitStack() as ph1:
                wpool = ph1.enter_context(tc.tile_pool(name="w", bufs=1))
                xqpool = ph1.enter_context(tc.tile_pool(name="xq", bufs=8))
                xpool = ph1.enter_context(tc.tile_pool(name="x", bufs=3))

                wq_r = wq_t.rearrange("(c p) j -> c p j", p=128).bitcast(F32R)
                wk_r = wk_t.rearrange("(c p) j -> c p j", p=128).bitcast(F32R)
                wv_r = wv_t.rearrange("(c p) j -> c p j", p=128).bitcast(F32R)
                xt_r = xt.rearrange("(c p) s -> c p s", p=128).bitcast(F32R)

                # stage the full first quarter: per ct interleave x, wq, wk
                # DMAs so the first pass never runs dry; wv is deferred until
                # after the QK passes (the V pass runs last).
                x_first = []
                wq_sb, wk_sb, wv_sb = [], [], []
                for ct in range(CT):
                    x_t = xqpool.tile([128, 512], F32R, tag="xq", name=f"xqk0{ct}")
                    nc.sync.dma_start(out=x_t, in_=xt_r[ct, :, 0:512])
                    x_first.append(x_t)
                    for lst, srct, nm in ((wq_sb, wq_r, "wq"), (wk_sb, wk_r, "wk")):
                        w = wpool.tile([128, CH], F32R, tag=f"{nm}{ct}",
                                       name=f"{nm}{ct}")
                        nc.sync.dma_start(out=w, in_=srct[ct])
                        lst.append(w)

                # Combined Q+K pass over query-range quarters (x streamed once).
                # PSUM tag t{j} holds Q_j in columns 0:512 and K_j in 512:1024.
                for sq in range(4):
                    pqk = [ppool.tile([128, 1024], F32, tag=f"t{j}",
                                      name=f"pqk{sq}{j}")
                           for j in range(JT)]
                    for ct in range(CT):
                        if sq == 0:
                            x_t = x_first[ct]
                        else:
                            x_t = xqpool.tile([128, 512], F32R, tag="xq",
                                              name=f"xqk{sq}_{ct}")
                            nc.sync.dma_start(
                                out=x_t,
                                in_=xt_r[ct, :, sq * 512:(sq + 1) * 512])
                        for j in range(JT):
                            nc.tensor.matmul(
                                pqk[j][:, 0:512],
                                lhsT=wq_sb[ct][:, j * 128:(j + 1) * 128],
                                rhs=x_t,
                                start=(ct == 0), stop=(ct == CT - 1))
                        for j in range(JT):
                            nc.tensor.matmul(
                                pqk[j][:, 512:1024],
                                lhsT=wk_sb[ct][:, j * 128:(j + 1) * 128],
                                rhs=x_t,
                                start=(ct == 0), stop=(ct == CT - 1))
                    for j in range(JT):
                        # drain each tag via three engines-worth of copies:
                        # q head-even on ACT, q head-odd + k on DVE
                        h0, h1 = 2 * j, 2 * j + 1
                        nc.scalar.activation(
                            qp_sb[h0][0:64, sq * 512:(sq + 1) * 512],
                            pqk[j][0:64, 0:512],
                            mybir.ActivationFunctionType.Identity,
                            bias=bqp[0:64, j:j + 1], scale=1.0)
                        nc.vector.tensor_scalar_add(
                            qp_sb[h1][64:128, sq * 512:(sq + 1) * 512],
                            pqk[j][64:128, 0:512],
                            bqp[64:128, j:j + 1])
                        nc.vector.tensor_scalar_add(
                            kt_sb[j][:, sq * 512:(sq + 1) * 512],
                            pqk[j][:, 512:1024],
                            bkp[:, j:j + 1])

                # V pass: tokens on psum partitions (x streamed a second time).
                for ct in range(CT):
                    w = wpool.tile([128, CH], F32R, tag=f"wv{ct}", name=f"wv{ct}")
                    nc.sync.dma_start(out=w, in_=wv_r[ct])
                    wv_sb.append(w)
                for sh in range(2):
                    pv = [ppool.tile([128, 1024], F32, tag=f"t{j}",
                                     name=f"pv{sh}{j}")
                          for j in range(JT)]
                    for ct in range(CT):
                        x_t = xpool.tile([128, 1024], F32R, tag="x",
                                         name=f"xv{sh}{ct}")
                        nc.sync.dma_start(
                            out=x_t, in_=xt_r[ct, :, sh * 1024:(sh + 1) * 1024])
                        for st in range(8):
                            nc.tensor.matmul(
                                pv[st // 2][:, (st % 2) * 512:(st % 2 + 1) * 512],
                                lhsT=x_t[:, st * 128:(st + 1) * 128],
                                rhs=wv_sb[ct],
                                start=(ct == 0), stop=(ct == CT - 1))
                    for st in range(8):
                        sl = pv[st // 2][:, (st % 2) * 512:(st % 2 + 1) * 512]
                        for h in range(HPC):
                            nc.vector.tensor_add(
                                v_sb[:, sh * 8 + st, h * VW:h * VW + HD],
                                sl[:, h * HD:(h + 1) * HD],
                                bv_bc[:, h * HD:(h + 1) * HD])

            # ---------------- Phase 2: attention ----------------
            with ExitStack() as ph2:
                epool = ph2.enter_context(tc.tile_pool(name="ep", bufs=8))
                opool = ph2.enter_context(tc.tile_pool(name="op", bufs=3))

                for h in range(HPC):
                    qi = h // 2
                    for ih in range(2):
                        blk = h * 2 + ih
                        i0 = ih * 1024
                        ctx_ps = ppool.tile([VW, 1024], F32, tag="t3",
                                            name=f"ctx{blk}")
                        for st in range(ST):
                            s_ps = ppool.tile([128, 1024], F32, tag=f"t{st % 3}",
                                              name=f"sc{blk}_{st}")
                            for q in range(2):
                                nc.tensor.matmul(
                                    s_ps[:, q * 512:(q + 1) * 512],
                                    lhsT=kt_sb[qi][:, st * 128:(st + 1) * 128],
                                    rhs=qp_sb[h][:, i0 + q * 512:i0 + (q + 1) * 512],
                                    start=True, stop=True)
                            e_sb = epool.tile([128, 1024], F32R, tag="e",
                                              name=f"e{blk}_{st}")
                            nc.scalar.activation(
                                e_sb, s_ps,
                                mybir.ActivationFunctionType.Exp,
                                bias=mask_sb[:, st:st + 1], scale=0.125)
                            for q in range(2):
                                nc.tensor.matmul(
                                    ctx_ps[:, q * 512:(q + 1) * 512],
                                    lhsT=v_sb[:, st, h * VW:(h + 1) * VW],
                                    rhs=e_sb[:, q * 512:(q + 1) * 512],
                                    start=(st == 0), stop=(st == ST - 1))
                        o_sb = opool.tile([VW, 1024], F32, tag="o", name=f"o{blk}")
                        nc.vector.tensor_copy(o_sb, ctx_ps)
                        nc.sync.dma_start(
                            out=out[h * VW:(h + 1) * VW, i0:i0 + 1024], in_=o_sb)

    nc.compile()
    return nc


def _get_nc():
    if "nc" not in _CACHE:
        _CACHE["nc"] = _build()
    return _CACHE["nc"]


def _in_maps(hidden_states, attention_mask, wq, bq, wk, bk, wv, bv):
    ones = np.ones(512, np.float32)
    maps = []
    for c in range(8):
        b, g = c // 2, c % 2
        ch0 = g * CH
        maps.append({
            "xt": np.ascontiguousarray(hidden_states[b].T),
            "wq_t": np.ascontiguousarray(wq[ch0:ch0 + CH, :].T),
            "wk_t": np.ascontiguousarray(wk[ch0:ch0 + CH, :].T),
            "wv_t": np.ascontiguousarray(wv[ch0:ch0 + CH, :].T),
            "bq": np.ascontiguousarray(bq[ch0:ch0 + CH]),
            "bk": np.ascontiguousarray(bk[ch0:ch0 + CH]),
            "bv": np.ascontiguousarray(bv[ch0:ch0 + CH]),
            "mask": np.ascontiguousarray(attention_mask[b, 0, 0, :]),
            "ones": ones,
        })
    return maps


def _gather(results):
    full = np.empty((B, S, H), np.float32)
    for c in range(8):
        b, g = c // 2, c % 2
        o = results[c]["out"].reshape(HPC, VW, S)
        ctx = o[:, :HD, :] / o[:, HD:HD + 1, :]        # normalize by denom row
        # [h, d, s] -> [s, h*d]
        full[b, :, g * CH:(g + 1) * CH] = ctx.reshape(CH, S).T
    return full


def _run(in_maps, trace=False):
    from concourse.bass_utils import run_bass_kernel_spmd

    nc = _get_nc()
    return run_bass_kernel_spmd(nc, in_maps, list(range(8)), trace=trace)


def _run_results(in_maps):
    """Run on hardware; on a wedged-device error retry in fresh subprocesses
    (the PJRT client cannot recover an unrecoverable exec unit in-process)."""
    try:
        return _run(in_maps).results
    except Exception:
        pass
    import pickle
    import subprocess
    import tempfile

    last = None
    for _ in range(3):
        try:
            with tempfile.TemporaryDirectory() as td:
                fin = os.path.join(td, "in.pkl")
                fout = os.path.join(td, "out.pkl")
                with open(fin, "wb") as f:
                    pickle.dump(in_maps, f)
                code = (
                    "import pickle, sys\n"
                    f"sys.path.insert(0, {_KERNEL_DIR!r})\n"
                    "import kernel\n"
                    f"maps = pickle.load(open({fin!r}, 'rb'))\n"
                    "res = kernel._run(maps)\n"
                    f"pickle.dump(res.results, open({fout!r}, 'wb'))\n"
                )
                subprocess.run([sys.executable, "-c", code], check=True,
                               timeout=1800)
                with open(fout, "rb") as f:
                    return pickle.load(f)
        except Exception as e:
            last = e
    raise last


def kernel(hidden_states, attention_mask, wq, bq, wk, bk, wv, bv):
    args = [np.asarray(a, np.float32) for a in
            (hidden_states, attention_mask, wq, bq, wk, bk, wv, bv)]
    return _gather(_run_results(_in_maps(*args)))


def kernel_profiled(hidden_states, attention_mask, wq, bq, wk, bk, wv, bv):
    """Like kernel() but with NTFF tracing; returns (output, exec_time_ns)."""
    args = [np.asarray(a, np.float32) for a in
            (hidden_states, attention_mask, wq, bq, wk, bk, wv, bv)]
    res = _run(_in_maps(*args), trace=True)
    return _gather(res.results), res.exec_time_ns


# revision 22
# speedup vs baseline: 1.1257x; 1.1257x over previous
"""BertSelfAttention (B=4, S=2048, H=1024, NH=16, HD=64) on 8 Trainium2 NeuronCores.

Sharding: batch (4) x head-group (2) -> 8 cores. Core c handles batch b=c//2 and
heads [g*8, g*8+8) with g=c%2 (output channels [g*512, (g+1)*512)).

Per-core math (all on device):
  QT[ch, s] = (wq_c @ x_b^T + bq_c),  KT likewise       (channels on partitions)
  V[s, ch]  = (x_b @ wv_c^T + bv_c)                     (tokens on partitions)
  per (head h, query half ih), per key tile st (128 keys j):
      scoresT[j, i] -> [128, 1024] PSUM (2 matmuls), ping-pong buffered
      expT = exp(scoresT/8 + mask_j)   (one ACT op; mask is per-partition bias)
      ctxT[d, i] += [v_h | 1]^T-weighted expT           (fused denominator row)
  Device emits unnormalized ctxT + denom rows [8*65, 2048]; the host divides and
  transposes into [B, S, H].

Two Trainium2-specific tricks matter here:
  * Changing the matmul contraction size (K) between back-to-back matmuls costs
    ~1.6us in PE reconfiguration, so every matmul keeps K=128: Q is stored
    per-head zero-padded to 128 partitions (the other head's K rows hit zeros),
    while KT stays packed two heads per tile.
  * All PSUM lives in one pool of 4 [128, 1024] tags: QKV passes use tile
    halves as 8 accumulators, attention ping-pongs scores on tags 0/1 and ctx
    on tags 2/3 -- no pool-transition barrier or head-boundary PE stalls.

Matmuls run as float32r (full-rate fp32 with hardware rounding, ~2e-4 rel err).
"""

import os
import sys

if "/opt/trn_rl_repo" not in sys.path:
    sys.path.insert(0, "/opt/trn_rl_repo")

import numpy as np

_KERNEL_DIR = os.path.dirname(os.path.abspath(__file__))

B, S, H = 4, 2048, 1024
NH, HD = 16, 64
HPC = 8          # heads per core
CH = HPC * HD    # 512 output channels per core
CT = H // 128    # 8 contraction tiles
JT = CH // 128   # 4 channel tiles per core
ST = S // 128    # 16 token tiles
VW = HD + 1      # 65: v columns + fused ones column

_CACHE = {}


def _build():
    import concourse.bass as bass  # noqa: F401  (registers engine methods)
    import concourse.mybir as mybir
    import concourse.tile as tile
    from concourse import bacc

    F32 = mybir.dt.float32
    F32R = mybir.dt.float32r

    nc = bacc.Bacc("TRN2", target_bir_lowering=False, debug=True)

    xt = nc.dram_tensor("xt", [H, S], F32, kind="ExternalInput")        # x_b^T
    wq_t = nc.dram_tensor("wq_t", [H, CH], F32, kind="ExternalInput")   # wq_c^T
    wk_t = nc.dram_tensor("wk_t", [H, CH], F32, kind="ExternalInput")
    wv_t = nc.dram_tensor("wv_t", [H, CH], F32, kind="ExternalInput")
    bq = nc.dram_tensor("bq", [CH], F32, kind="ExternalInput")
    bk = nc.dram_tensor("bk", [CH], F32, kind="ExternalInput")
    bv = nc.dram_tensor("bv", [CH], F32, kind="ExternalInput")
    mask = nc.dram_tensor("mask", [S], F32, kind="ExternalInput")
    ones = nc.dram_tensor("ones", [512], F32, kind="ExternalInput")
    # unnormalized ctxT + denominator rows, 65 rows per head
    out = nc.dram_tensor("out", [VW * HPC, S], F32, kind="ExternalOutput")

    with tile.TileContext(nc) as tc, nc.allow_low_precision(reason="fp32r attention"):
        from contextlib import ExitStack

        with ExitStack() as outer:
            persist = outer.enter_context(tc.tile_pool(name="persist", bufs=1))
            ppool = outer.enter_context(tc.tile_pool(name="pp", bufs=1, space="PSUM"))

            # Persistent SBUF tensors
            # Q per head, zero-padded to 128 partitions (head h lives in its own
            # partition range po:po+64; the other 64 rows are zeros).
            qp_sb = [persist.tile([128, S], F32R, tag=f"qp{h}", name=f"qp{h}")
                     for h in range(HPC)]
            kt_sb = [persist.tile([128, S], F32R, tag=f"kt{j}", name=f"kt{j}")
                     for j in range(JT)]
            v_sb = persist.tile([128, ST, VW * HPC], F32R, tag="v")
            mask_sb = persist.tile([128, ST], F32, tag="mask")
            bqp = persist.tile([128, JT], F32, tag="bqp")
            bkp = persist.tile([128, JT], F32, tag="bkp")
            bv_bc = persist.tile([128, CH], F32, tag="bv_bc")
            ones8 = persist.tile([128, HPC], F32R, tag="ones8")
            zcol = persist.tile([128, 1], F32, tag="zcol")

            nc.sync.dma_start(out=mask_sb, in_=mask.rearrange("(t p) -> p t", p=128))
            nc.sync.dma_start(out=bqp, in_=bq.rearrange("(j p) -> p j", p=128))
            nc.sync.dma_start(out=bkp, in_=bk.rearrange("(j p) -> p j", p=128))
            nc.sync.dma_start(
                out=bv_bc,
                in_=bass.AP(tensor=bv, offset=0, ap=[[0, 128], [1, CH]]))
            nc.sync.dma_start(
                out=ones8,
                in_=bass.AP(tensor=ones.bitcast(F32R), offset=0,
                            ap=[[0, 128], [1, HPC]]))
            # ones columns of v (position 64 of each head block, every token tile)
            v4 = v_sb.rearrange("p t (h e) -> p t h e", e=VW)
            for t in range(ST):
                nc.vector.tensor_copy(v4[:, t, :, HD], ones8)
            # zero the unused partition half of each padded-Q tile
            nc.vector.memset(zcol, 0.0)
            for h in range(HPC):
                zo = 64 if h % 2 == 0 else 0      # rows NOT owned by head h
                zsrc = zcol[zo:zo + 64, 0:1]
                zbcast = bass.AP(tensor=zsrc.tensor, offset=zsrc.offset,
                                 ap=[zsrc.ap[0], [0, S]])
                nc.vector.tensor_copy(qp_sb[h][zo:zo + 64, :], zbcast)

            # ---------------- Phase 1: QKV projections ----------------
            with ExitStack() as ph1:
                wpool = ph1.enter_context(tc.tile_pool(name="w", bufs=1))
                xqpool = ph1.enter_context(tc.tile_pool(name="xq", bufs=8))
                xpool = ph1.enter_context(tc.tile_pool(name="x", bufs=3))

                wq_r = wq_t.rearrange("(c p) j -> c p j", p=128).bitcast(F32R)
                wk_r = wk_t.rearrange("(c p) j -> c p j", p=128).bitcast(F32R)
                wv_r = wv_t.rearrange("(c p) j -> c p j", p=128).bitcast(F32R)
                xt_r = xt.rearrange("(c p) s -> c p s", p=128).bitcast(F32R)

                # stage the full first quarter: per ct interleave x, wq, wk
                # DMAs so the first pass never runs dry; wv is deferred until
                # after the QK passes (the V pass runs last).
                x_first = []
                wq_sb, wk_sb, wv_sb = [], [], []
                for ct in range(CT):
                    x_t = xqpool.tile([128, 512], F32R, tag="xq", name=f"xqk0{ct}")
                    nc.sync.dma_start(out=x_t, in_=xt_r[ct, :, 0:512])
                    x_first.append(x_t)
                    for lst, srct, nm in ((wq_sb, wq_r, "wq"), (wk_sb, wk_r, "wk")):
                        w = wpool.tile([128, CH], F32R, tag=f"{nm}{ct}",
                                       name=f"{nm}{ct}")
                        nc.sync.dma_start(out=w, in_=srct[ct])
                        lst.append(w)

                # Combined Q+K pass over query-range quarters (x streamed once).
                # PSUM tag t{j} holds Q_j in columns 0:512 and K_j in 512:1024.
                for sq in range(4):
                    pqk = [ppool.tile([128, 1024], F32, tag=f"t{j}",
                                      name=f"pqk{sq}{j}")
                           for j in range(JT)]
                    for ct in range(CT):
                        if sq == 0:
                            x_t = x_first[ct]
                        else:
                            x_t = xqpool.tile([128, 512], F32R, tag="xq",
                                              name=f"xqk{sq}_{ct}")
                            nc.sync.dma_start(
                                out=x_t,
                                in_=xt_r[ct, :, sq * 512:(sq + 1) * 512])
                        for j in range(JT):
                            nc.tensor.matmul(
                                pqk[j][:, 0:512],
                                lhsT=wq_sb[ct][:, j * 128:(j + 1) * 128],
                                rhs=x_t,
                                start=(ct == 0), stop=(ct == CT - 1))
                        for j in range(JT):
                            nc.tensor.matmul(
                                pqk[j][:, 512:1024],
                                lhsT=wk_sb[ct][:, j * 128:(j + 1) * 128],
                                rhs=x_t,
                                start=(ct == 0), stop=(ct == CT - 1))
                    for j in range(JT):
                        # drain each tag via three engines-worth of copies:
                        # q head-even on ACT, q head-odd + k on DVE
                        h0, h1 = 2 * j, 2 * j + 1
                        nc.scalar.activation(
                            qp_sb[h0][0:64, sq * 512:(sq + 1) * 512],
                            pqk[j][0:64, 0:512],
                            mybir.ActivationFunctionType.Identity,
                            bias=bqp[0:64, j:j + 1], scale=1.0)
                        nc.vector.tensor_scalar_add(
                            qp_sb[h1][64:128, sq * 512:(sq + 1) * 512],
                            pqk[j][64:128, 0:512],
                            bqp[64:128, j:j + 1])
                        nc.vector.tensor_scalar_add(
                            kt_sb[j][:, sq * 512:(sq + 1) * 512],
                            pqk[j][:, 512:1024],
                            bkp[:, j:j + 1])

                # V pass: tokens on psum partitions (x streamed a second time).
                for ct in range(CT):
                    w = wpool.tile([128, CH], F32R, tag=f"wv{ct}", name=f"wv{ct}")
                    nc.sync.dma_start(out=w, in_=wv_r[ct])
                    wv_sb.append(w)
                for sh in range(2):
                    pv = [ppool.tile([128, 1024], F32, tag=f"t{j}",
                                     name=f"pv{sh}{j}")
                          for j in range(JT)]
                    for ct in range(CT):
                        x_t = xpool.tile([128, 1024], F32R, tag="x",
                                         name=f"xv{sh}{ct}")
                        nc.sync.dma_start(
                            out=x_t, in_=xt_r[ct, :, sh * 1024:(sh + 1) * 1024])
                        for st in range(8):
                            nc.tensor.matmul(
                                pv[st // 2][:, (st % 2) * 512:(st % 2 + 1) * 512],
                                lhsT=x_t[:, st * 128:(st + 1) * 128],
                                rhs=wv_sb[ct],
                                start=(ct == 0), stop=(ct == CT - 1))
                    for st in range(8):
                        sl = pv[st // 2][:, (st % 2) * 512:(st % 2 + 1) * 512]
                        for h in range(HPC):
                            nc.vector.tensor_add(
                                v_sb[:, sh * 8 + st, h * VW:h * VW + HD],
                                sl[:, h * HD:(h + 1) * HD],
                                bv_bc[:, h * HD:(h + 1) * HD])

            # ---------------- Phase 2: attention ----------------
            with ExitStack() as ph2:
                epool = ph2.enter_context(tc.tile_pool(name="ep", bufs=8))
                opool = ph2.enter_context(tc.tile_pool(name="op", bufs=3))

                for h in range(HPC):
                    qi = h // 2
                    for ih in range(2):
                        blk = h * 2 + ih
                        i0 = ih * 1024
                        ctx_ps = ppool.tile([VW, 1024], F32, tag=f"t{2 + blk % 2}",
                                            name=f"ctx{blk}")
                        for st in range(ST):
                            s_ps = ppool.tile([128, 1024], F32, tag=f"t{st % 2}",
                                              name=f"sc{blk}_{st}")
                            for q in range(2):
                                nc.tensor.matmul(
                                    s_ps[:, q * 512:(q + 1) * 512],
                                    lhsT=kt_sb[qi][:, st * 128:(st + 1) * 128],
                                    rhs=qp_sb[h][:, i0 + q * 512:i0 + (q + 1) * 512],
                                    start=True, stop=True)
                            e_sb = epool.tile([128, 1024], F32R, tag="e",
                                              name=f"e{blk}_{st}")
                            nc.scalar.activation(
                                e_sb, s_ps,
                                mybir.ActivationFunctionType.Exp,
                                bias=mask_sb[:, st:st + 1], scale=0.125)
                            for q in range(2):
                                nc.tensor.matmul(
                                    ctx_ps[:, q * 512:(q + 1) * 512],
                                    lhsT=v_sb[:, st, h * VW:(h + 1) * VW],
                                    rhs=e_sb[:, q * 512:(q + 1) * 512],
                                    start=(st == 0), stop=(st == ST - 1))
                        o_sb = opool.tile([VW, 1024], F32, tag="o", name=f"o{blk}")
                        nc.vector.tensor_copy(o_sb, ctx_ps)
                        nc.sync.dma_start(
                            out=out[h * VW:(h + 1) * VW, i0:i0 + 1024], in_=o_sb)

    nc.compile()
    return nc


def _get_nc():
    if "nc" not in _CACHE:
        _CACHE["nc"] = _build()
    return _CACHE["nc"]


def _in_maps(hidden_states, attention_mask, wq, bq, wk, bk, wv, bv):
    ones = np.ones(512, np.float32)
    maps = []
    for c in range(8):
        b, g = c // 2, c % 2
        ch0 = g * CH
        maps.append({
            "xt": np.ascontiguousarray(hidden_states[b].T),
            "wq_t": np.ascontiguousarray(wq[ch0:ch0 + CH, :].T),
            "wk_t": np.ascontiguousarray(wk[ch0:ch0 + CH, :].T),
            "wv_t": np.ascontiguousarray(wv[ch0:ch0 + CH, :].T),
            "bq": np.ascontiguousarray(bq[ch0:ch0 + CH]),
            "bk": np.ascontiguousarray(bk[ch0:ch0 + CH]),
            "bv": np.ascontiguousarray(bv[ch0:ch0 + CH]),
            "mask": np.ascontiguousarray(attention_mask[b, 0, 0, :]),
            "ones": ones,
        })
    return maps


def _gather(results):
    full = np.empty((B, S, H), np.float32)
    for c in range(8):
        b, g = c // 2, c % 2
        o = results[c]["out"].reshape(HPC, VW, S)
        ctx = o[:, :HD, :] / o[:, HD:HD + 1, :]        # normalize by denom row
        # [h, d, s] -> [s, h*d]
        full[b, :, g * CH:(g + 1) * CH] = ctx.reshape(CH, S).T
    return full


def _run(in_maps, trace=False):
    from concourse.bass_utils import run_bass_kernel_spmd

    nc = _get_nc()
    return run_bass_kernel_spmd(nc, in_maps, list(range(8)), trace=trace)


def _run_results(in_maps):
    """Run on hardware; on a wedged-device error retry in fresh subprocesses
    (the PJRT client cannot recover an unrecoverable exec unit in-process)."""
    try:
        return _run(in_maps).results
    except Exception:
        pass
    import pickle
    import subprocess
    import tempfile

    last = None
    for _ in range(3):
        try:
            with tempfile.TemporaryDirectory() as td:
                fin = os.path.join(td, "in.pkl")
                fout = os.path.join(td, "out.pkl")
                with open(fin, "wb") as f:
                    pickle.dump(in_maps, f)
                code = (
                    "import pickle, sys\n"
                    f"sys.path.insert(0, {_KERNEL_DIR!r})\n"
                    "import kernel\n"
                    f"maps = pickle.load(open({fin!r}, 'rb'))\n"
                    "res = kernel._run(maps)\n"
                    f"pickle.dump(res.results, open({fout!r}, 'wb'))\n"
                )
                subprocess.run([sys.executable, "-c", code], check=True,
                               timeout=1800)
                with open(fout, "rb") as f:
                    return pickle.load(f)
        except Exception as e:
            last = e
    raise last


def kernel(hidden_states, attention_mask, wq, bq, wk, bk, wv, bv):
    args = [np.asarray(a, np.float32) for a in
            (hidden_states, attention_mask, wq, bq, wk, bk, wv, bv)]
    return _gather(_run_results(_in_maps(*args)))


def kernel_profiled(hidden_states, attention_mask, wq, bq, wk, bk, wv, bv):
    """Like kernel() but with NTFF tracing; returns (output, exec_time_ns)."""
    args = [np.asarray(a, np.float32) for a in
            (hidden_states, attention_mask, wq, bq, wk, bk, wv, bv)]
    res = _run(_in_maps(*args), trace=True)
    return _gather(res.results), res.exec_time_ns
